# revision 48
# baseline (speedup 1.0000x reference)
"""CNF vector-field + exact Jacobian-trace kernel for Trainium2 (8 NeuronCores).

Math: for each sample x (D=32), with inp = [x, t] (33,):
  h1 = tanh(inp @ W1 + b1); h2 = tanh(h1 @ W2 + b2); dx = h2 @ W3 + b3
  div = trace(J) = d1^T C d2,  C = W2 * (W3 @ W1r)^T,  d_i = 1 - h_i^2
  out = [dx, div]  (B, 33)

v2 implementation notes (vs the 33.6us baseline):
  - data-parallel over batch (2048 -> 8 x 256), weights replicated
  - host precomputes negmt = -(W3 @ W1r)^T (weight-only): kills the on-device
    W3-transpose + negMt matmul chain and 4 PSUM banks
  - P = -C = w2k * negmt chunk (DVE);  gt_m = sum_k P_k[:,m]^T [h1sq_k | 1]
    -> col 256 of each gt bank is vP chunk; g = raw - col; E = (h2sq-1)*g;
    div = (-1)^T E.  No vP row matmuls (saved 8 N=512/256 MMs)
  - w2 / negmt / h1t / pmat / h1sq travel in bf16 (W2 DMA halves; matmuls
    run 1 col/cycle regardless); x/W1/z1/h2t/dx/div path stays f32(r)
  - colp/rowp DMAs are gone (the [128,11] colp DMA alone took 2.5us to
    issue): bias columns ride in the x DMA, b3 row rides in x too,
    ones/-1 columns are GpSimd memsets
  - DMA queues: SP carries xaug then w2; ACT carries w1, negmt, w3; the
    issue serialisation that delayed w2's completion to 17.8us now has w2
    landing ~12.5us
  - 9 back-to-back spam matmuls on scratch SBUF warm the PE HAM clock-gate
    (4/8 -> 8/8) during the DMA wait, so the real matmul burst runs at
    2.4GHz instead of 1.2GHz (baseline flipped only at 22.5us of 29.5)
  - emission order = engine-stream order; z2 k-outer consumes h1t as tanh1
    lands, gt m-outer staggers PSUM-stop so the DVE t/e tail pipelines with
    the div matmuls
"""
import sys

for _p in ("/opt/trn_rl_repo", "/root/.axon_site/_ro/trn_rl_repo"):
    if _p not in sys.path:
        sys.path.append(_p)

import numpy as np
import ml_dtypes

B, D, H = 2048, 32, 512
NCORES = 8
BC = B // NCORES          # 256 rows per core
NK = H // 128             # 4 chunks of the hidden dim
# allin column map: one consolidated [128, AC] f32 input DMA carrying
# everything except the two big bf16 matrices. Small separate DMAs are
# poison here: few-descriptor DMAs (w1 as [34,512]) complete ~4us late,
# and each extra dma_start risks a ~4us issue stall.
XC = 112                  # 0:66 x halves | 66:70 bias1 | 70:74 b2 | 74:106 b3row
W1C = 112                 # 112:624  W1r on partitions 0:32
W3C0 = 624                # 624:752  w3 lhsT chunks (4 x 32)
AC = 768                  # padded total (3KB lines)

_CACHE = {}


def _build():
    import concourse.bass as bass
    import concourse.tile as tile
    from concourse import bacc, mybir
    from concourse.masks import make_identity

    f32 = mybir.dt.float32
    f32r = mybir.dt.float32r
    bf16 = mybir.dt.bfloat16
    AF = mybir.ActivationFunctionType
    ALU = mybir.AluOpType

    nc = bacc.Bacc("TRN2", target_bir_lowering=False, debug=False,
                   num_devices=NCORES)

    # consolidated input (see column map above)
    allin_ext = nc.dram_tensor("allin", [128, AC], f32r, kind="ExternalInput").ap()
    w2_ext = nc.dram_tensor("w2", [H, H], bf16, kind="ExternalInput").ap()
    # negc = -(W2 * (W3 @ W1r)^T)  (H, H): P itself, host-precomputed
    negc_ext = nc.dram_tensor("negc", [H, H], bf16, kind="ExternalInput").ap()
    out_ext = nc.dram_tensor("out", [BC, D + 1], f32, kind="ExternalOutput").ap()

    with tile.TileContext(nc) as tc:
        with tc.tile_pool(name="const", bufs=1) as cpool, \
             tc.tile_pool(name="work", bufs=1) as wpool, \
             tc.tile_pool(name="ps", bufs=1, space="PSUM") as pps:

            def big_ps(nm):
                return pps.tile([128, 288], f32, name=nm, tag="big", bufs=6)

            def small_ps(nm, shape):
                return pps.tile(shape, f32, name=nm, tag="small", bufs=2)

            # -------- ACT spline-table preload (overlaps the DMA phase) ----
            dm0 = wpool.tile([1, 1], f32, name="dm0")
            dm1 = wpool.tile([1, 1], f32, name="dm1")
            nc.gpsimd.memset(dm0[:, :], 0.0)
            nc.scalar.activation(dm1[:, :], dm0[:, :], AF.Tanh)

            # ------------- input DMAs -------------
            # ACT queue: the consolidated allin (gates the whole z1 chain).
            # NOTE: splitting allin across both rings lands it 1.3us
            # earlier but makes the early PE schedule sparser, which trips
            # a HAM mid-burst re-throttle — measured net LOSS (28330 vs
            # 27922). The dense-late schedule wins.
            allin = cpool.tile([128, AC], f32r, name="allin")
            nc.scalar.dma_start(out=allin[:, :], in_=allin_ext[:, :])
            xat = allin
            w3k = [allin[:, W3C0 + k * D:W3C0 + (k + 1) * D] for k in range(NK)]
            b3row = allin[0:1, 74:106]
            bias1c = [allin[:, 66 + m:67 + m] for m in range(NK)]
            b2c = [allin[:, 70 + m:71 + m] for m in range(NK)]

            # negc rides the ACT ring BEHIND allin: same-row descriptors
            # drain FIFO, so allin gets full share of the SDMA round-robin
            # against w2 (sync row) and lands first; negc (needed latest,
            # by gt) drains after.
            pall = cpool.tile([128, NK * H], bf16, name="pall")
            nc.scalar.dma_start(
                out=pall[:, :].rearrange("p (k j) -> p k j", k=NK),
                in_=negc_ext.rearrange("(k p) j -> p k j", k=NK))
            pmat = [pall[:, k * H:(k + 1) * H] for k in range(NK)]

            # SP queue: w2 alone (z2 comes first in the PE stream)
            w2all = cpool.tile([128, NK * H], bf16, name="w2all")
            nc.sync.dma_start(
                out=w2all[:, :].rearrange("p (k j) -> p k j", k=NK),
                in_=w2_ext.rearrange("(k p) j -> p k j", k=NK))
            w2k = [w2all[:, k * H:(k + 1) * H] for k in range(NK)]

            # ------------- constants via memset (+DVE copy for f32r) -------
            scratch = wpool.tile([128, H], f32, name="scratch")
            nc.gpsimd.memset(scratch[:, :], 0.125)
            scratchr = wpool.tile([128, H], f32r, name="scratchr")
            nc.vector.tensor_copy(scratchr[:, :], scratch[:, :])
            negc0 = wpool.tile([128, 1], f32, name="negc0")
            nc.gpsimd.memset(negc0[:, :], -1.0)
            negcol = wpool.tile([128, 1], f32r, name="negcol")
            nc.vector.tensor_copy(negcol[:, :], negc0[:, :])
            ones0 = wpool.tile([1, BC], f32, name="ones0")
            nc.gpsimd.memset(ones0[:, :], 1.0)
            ones_row = wpool.tile([1, BC], f32r, name="ones_row")
            nc.vector.tensor_copy(ones_row[:, :], ones0[:, :])

            # h1sq tiles carry an appended ones column (col 256)
            h1sq = [wpool.tile([128, BC + 1], bf16, name=f"h1sq_{k}")
                    for k in range(NK)]
            for k in range(NK):
                nc.gpsimd.memset(h1sq[k][:, BC:BC + 1], 1.0)

            ident = cpool.tile([128, 128], f32, name="ident")
            make_identity(nc, ident[:, :])

            # -------- PE HAM warm-up: dependency-free spam matmuls --------
            # 5 f32r N=512 MMs (~2.1us) bridge the PE into the xT/z1/z2
            # stream; the combined continuous activity flips the HAM
            # clock-gate (4/8 -> 8/8) partway into the z2 burst
            spam_ps = small_ps("spam", [128, H])
            for _ in range(7):
                nc.tensor.matmul(spam_ps[:, :], scratchr[:, 0:128],
                                 scratchr[:, :], start=True, stop=True)

            # ---------------- x transpose: a0 = xs^T (32, 256) -------------
            a0 = wpool.tile([D, BC], f32r, name="a0")
            for i in range(2):
                xp = small_ps("xT", [D + 1, 128])
                nc.tensor.transpose(xp[:, :],
                                    xat[:, i * (D + 1):(i + 1) * (D + 1)].bitcast(f32),
                                    ident[:, :])
                nc.vector.tensor_copy(a0[:, i * 128:(i + 1) * 128], xp[0:D, :])

            # ---------------- layer 1: z1 -> tanh -> h1sq ----------------
            z1s = []
            for m in range(NK):
                z1 = big_ps("z1")
                nc.tensor.matmul(z1[:, 0:BC],
                                 allin[0:D, W1C + m * 128:W1C + (m + 1) * 128],
                                 a0[:, :], start=True, stop=True)
                z1s.append(z1)
            h1t = []
            for m in range(NK):
                h = wpool.tile([128, BC], bf16, name=f"h1t_{m}")
                nc.scalar.activation(h[:, :], z1s[m][:, 0:BC], AF.Tanh,
                                     bias=bias1c[m].bitcast(f32))
                h1t.append(h)
            for m in range(NK):
                nc.vector.tensor_tensor(out=h1sq[m][:, 0:BC], in0=h1t[m][:, :],
                                        in1=h1t[m][:, :], op=ALU.mult)

            # ------- layer 2 + Jacobian chain, pipelined per m-chunk -------
            # z2 runs k-outer for k<3 (consuming h1t as tanh1 lands), then
            # per m: z2[m]'s last k-step stops the bank, tanh2[m] starts on
            # ACT while the PE immediately runs gt[m]'s 4 matmuls; the
            # DVE t/e chain and GpSimd h2sq follow per-m so the div inputs
            # stream out instead of bunching at the end.
            # gt_m = sum_k P_k[:,m]^T [h1sq_k | 1]; col BC = vP chunk;
            # g = raw - col; E = (h2sq-1)*g; div = (-1)^T E
            z2s = [big_ps("z2") for _ in range(NK)]
            for k in range(NK - 1):
                for m in range(NK):
                    nc.tensor.matmul(z2s[m][:, 0:BC],
                                     w2k[k][:, m * 128:(m + 1) * 128],
                                     h1t[k][:, :],
                                     start=(k == 0), stop=False)
            h2t, h2sq, vcol, tm, ee = [], [], [], [], []
            for m in range(NK):
                nc.tensor.matmul(z2s[m][:, 0:BC],
                                 w2k[NK - 1][:, m * 128:(m + 1) * 128],
                                 h1t[NK - 1][:, :], start=False, stop=True)
                h = wpool.tile([128, BC], f32r, name=f"h2t_{m}")
                nc.scalar.activation(h[:, :], z2s[m][:, 0:BC], AF.Tanh,
                                     bias=b2c[m].bitcast(f32))
                h2t.append(h)
                sq = wpool.tile([128, BC], f32, name=f"h2sq_{m}")
                nc.gpsimd.tensor_tensor(out=sq[:, :], in0=h[:, :].bitcast(f32),
                                        in1=h[:, :].bitcast(f32), op=ALU.mult)
                h2sq.append(sq)
                gt = big_ps("gt")
                for k in range(NK):
                    nc.tensor.matmul(gt[:, 0:BC + 1],
                                     pmat[k][:, m * 128:(m + 1) * 128],
                                     h1sq[k][:, :],
                                     start=(k == 0), stop=(k == NK - 1))
                vc = wpool.tile([128, 1], f32, name=f"vc_{m}")
                nc.vector.tensor_copy(vc[:, :], gt[:, BC:BC + 1])
                vcol.append(vc)
                t = wpool.tile([128, BC], f32r, name=f"t_{m}")
                nc.vector.tensor_scalar(out=t[:, :], in0=gt[:, 0:BC],
                                        scalar1=vc[:, :], scalar2=None,
                                        op0=ALU.subtract)
                tm.append(t)
                e = wpool.tile([128, BC], f32r, name=f"e_{m}")
                nc.vector.scalar_tensor_tensor(out=e[:, :], in0=sq[:, :],
                                               scalar=1.0, in1=t[:, :],
                                               op0=ALU.subtract, op1=ALU.mult)
                ee.append(e)

            # -------- dx = W3^T h2 + b3 ; div = (-1)^T E --------
            dx_ps = small_ps("dx_ps", [D, BC])
            for k in range(NK):
                nc.tensor.matmul(dx_ps[:, :], w3k[k], h2t[k][:, :],
                                 start=(k == 0), stop=False)
            nc.tensor.matmul(dx_ps[:, :], b3row, ones_row[:, :],
                             start=False, stop=True)
            outt = wpool.tile([D + 1, BC], f32, name="outt")
            nc.scalar.activation(outt[0:D, :], dx_ps[:, :], AF.Copy)
            div_ps = small_ps("div_ps", [1, BC])
            for m in range(NK):
                nc.tensor.matmul(div_ps[:, :], negcol[:, :], ee[m][:, :],
                                 start=(m == 0), stop=(m == NK - 1))
            nc.scalar.activation(outt[D:D + 1, :], div_ps[:, :], AF.Copy)

            # ------- transpose back to (256, 33) and store -------
            outs = wpool.tile([128, 2 * (D + 1)], f32, name="outs")
            for i in range(2):
                op = small_ps("outP", [128, D + 1])
                nc.tensor.transpose(op[:, :], outt[:, i * 128:(i + 1) * 128],
                                    ident[0:D + 1, 0:D + 1])
                nc.scalar.activation(outs[:, i * (D + 1):(i + 1) * (D + 1)],
                                     op[:, :], AF.Copy)
            nc.scalar.dma_start(
                out=out_ext.rearrange("(i p) c -> p i c", i=2),
                in_=outs[:, :].rearrange("p (i c) -> p i c", i=2))

    nc.compile()
    return nc


def _get_nc():
    if "nc" not in _CACHE:
        _CACHE["nc"] = _build()
    return _CACHE["nc"]


def _prep_inputs(t, x, W1, b1, W2, b2, W3, b3):
    t = np.asarray(t, dtype=np.float32)
    x = np.ascontiguousarray(np.asarray(x, dtype=np.float32))
    W1 = np.asarray(W1, dtype=np.float32)
    b1 = np.asarray(b1, dtype=np.float32)
    W2 = np.asarray(W2, dtype=np.float32)
    W3 = np.asarray(W3, dtype=np.float32)
    b3 = np.asarray(b3, dtype=np.float32)

    w2b = np.ascontiguousarray(W2.astype(ml_dtypes.bfloat16))

    Mt = (W3 @ W1[:D]).T.astype(np.float32)          # (H, H), Mt[a, j]
    negc = np.ascontiguousarray((-(W2 * Mt)).astype(ml_dtypes.bfloat16))

    bias1 = (np.float32(t.ravel()[0]) * W1[D, :] + b1).astype(np.float32)
    b2a = np.asarray(b2, dtype=np.float32)

    tmpl = np.zeros((128, AC), dtype=np.float32)
    tmpl[:, 66:70] = bias1.reshape(NK, 128).T
    tmpl[:, 70:74] = b2a.reshape(NK, 128).T
    tmpl[0, 74:106] = b3
    tmpl[0:D, W1C:W1C + H] = W1[:D]
    tmpl[:, W3C0:W3C0 + 128] = \
        W3.reshape(NK, 128, D).transpose(1, 0, 2).reshape(128, 128)

    xas = []
    for i in range(NCORES):
        xa = tmpl.copy()
        xc = x[i * BC:(i + 1) * BC]
        xa[:, 0:D + 1] = xc[0:128]
        xa[:, D + 1:2 * (D + 1)] = xc[128:256]
        xas.append(xa)
    return xas, w2b, negc


def kernel(t, x, W1, b1, W2, b2, W3, b3):
    from concourse.bass_utils import run_bass_kernel_spmd

    nc = _get_nc()
    xas, w2b, negc = _prep_inputs(t, x, W1, b1, W2, b2, W3, b3)
    in_maps = []
    for i in range(NCORES):
        in_maps.append({
            "allin": xas[i], "w2": w2b, "negc": negc,
        })
    res = run_bass_kernel_spmd(nc, in_maps, core_ids=list(range(NCORES)))
    return np.concatenate([res.results[i]["out"] for i in range(NCORES)], axis=0)
